# revision 11
# baseline (speedup 1.0000x reference)
import os
import sys

if "/opt/trn_rl_repo" not in sys.path:
    sys.path.insert(0, "/opt/trn_rl_repo")

import numpy as np
import ml_dtypes
from contextlib import ExitStack

import concourse.tile as tile
from concourse import bacc, mybir
from concourse import bass_utils

F32 = mybir.dt.float32
BF16 = mybir.dt.bfloat16
AF = mybir.ActivationFunctionType
ALU = mybir.AluOpType
AX = mybir.AxisListType
NPBF16 = ml_dtypes.bfloat16

B, C, L = 32, 128, 8192
N_CORES = 8
NB = B // N_CORES          # batches per core
CQ = C // 4
EPS = 1e-5
UCH = 1024                 # u-chunk width (PSUM fp32, 2 banks)
NCH = L // UCH             # u-chunks per batch
DCH = 2048                 # input DMA chunk width
OT = 512                   # p3 matmul tile (1 PSUM bank)
OG = 1024                  # output DMA group width

_BUILD_CACHE = {}


def _build(reps=1, loop_reps=0):
    key = (reps, loop_reps)
    if key in _BUILD_CACHE:
        return _BUILD_CACHE[key]

    nc = bacc.Bacc("TRN2", target_bir_lowering=False, debug=False)

    x_ap = nc.dram_tensor("x_dram", [NB, C, L], BF16, kind="ExternalInput").ap()
    w_u_ap = nc.dram_tensor("w_u", [C, C], BF16, kind="ExternalInput").ap()
    wsc_aps = [nc.dram_tensor(f"wsc{k}", [C, C], BF16, kind="ExternalInput").ap() for k in range(3)]
    w2t_ap = nc.dram_tensor("w2t", [C, C], BF16, kind="ExternalInput").ap()
    wfc1_ap = nc.dram_tensor("wfc1", [C, CQ], F32, kind="ExternalInput").ap()
    b1e_ap = nc.dram_tensor("b1e", [CQ, 1], F32, kind="ExternalInput").ap()
    wfc2_ap = nc.dram_tensor("wfc2", [CQ, C], F32, kind="ExternalInput").ap()
    b2_ap = nc.dram_tensor("b2", [C, 1], F32, kind="ExternalInput").ap()
    t2_ap = nc.dram_tensor("t2", [C, 1], F32, kind="ExternalInput").ap()
    wam_ap = nc.dram_tensor("wam", [C, C], F32, kind="ExternalInput").ap()
    wax_ap = nc.dram_tensor("wax", [C, C], F32, kind="ExternalInput").ap()
    out_ap = nc.dram_tensor("out_dram", [NB, C, L], BF16, kind="ExternalOutput").ap()

    env = os.environ.get
    xr_bufs = int(env("K_XRBUFS", "4"))
    x1_bufs = int(env("K_X1BUFS", "3"))
    up_bufs = int(env("K_UPBUFS", "3"))
    m_bufs = int(env("K_MBUFS", "3"))
    ub = int(env("K_UBUFS", "2"))
    ob = int(env("K_OBUFS", "3"))
    um2_eng = env("K_UM2ENG", "gpsimd")     # gpsimd | dve
    abs_eng = env("K_ABSENG", "dve")        # dve | act

    with tile.TileContext(nc) as tc, ExitStack() as ctx:
        wpool = ctx.enter_context(tc.tile_pool(name="wpool", bufs=1))
        xr_pool = ctx.enter_context(tc.tile_pool(name="xr", bufs=xr_bufs))
        x1_pool = ctx.enter_context(tc.tile_pool(name="x1", bufs=x1_bufs))
        up_pool = ctx.enter_context(tc.tile_pool(name="up", bufs=up_bufs))
        m_pool = ctx.enter_context(tc.tile_pool(name="mtile", bufs=m_bufs))
        junk_pool = ctx.enter_context(tc.tile_pool(name="junk", bufs=2))
        gj_pool = ctx.enter_context(tc.tile_pool(name="gjunk", bufs=2))
        out_pool = ctx.enter_context(tc.tile_pool(name="ot", bufs=3))
        st_pool = ctx.enter_context(tc.tile_pool(name="stats", bufs=2))
        row_pool = ctx.enter_context(tc.tile_pool(name="rows", bufs=2))
        w2a_pool = ctx.enter_context(tc.tile_pool(name="w2a", bufs=3))
        u_psp = ctx.enter_context(tc.tile_pool(name="u_ps", bufs=ub, space="PSUM"))
        o_psp = ctx.enter_context(tc.tile_pool(name="o_ps", bufs=ob, space="PSUM"))
        s_psp = ctx.enter_context(tc.tile_pool(name="s_ps", bufs=int(env("K_SBUFS", "1")), space="PSUM"))

        # ---- load weights (once) ----
        def wload(nm, ap, shape, dt):
            t = wpool.tile(shape, dt, tag=nm)
            nc.sync.dma_start(t[:], ap[:])
            return t

        w_u_t = wload("w_u_t", w_u_ap, [C, C], BF16)
        wsc_t = [wload(f"wsc{k}_t", wsc_aps[k], [C, C], BF16) for k in range(3)]
        w2t_t = wload("w2t_t", w2t_ap, [C, C], BF16)
        wfc1_t = wload("wfc1_t", wfc1_ap, [C, CQ], F32)
        b1e_t = wload("b1e_t", b1e_ap, [CQ, 1], F32)
        wfc2_t = wload("wfc2_t", wfc2_ap, [CQ, C], F32)
        b2_t = wload("b2_t", b2_ap, [C, 1], F32)
        t2_t = wload("t2_t", t2_ap, [C, 1], F32)
        wam_t = wload("wam_t", wam_ap, [C, C], F32)
        wax_t = wload("wax_t", wax_ap, [C, C], F32)
        ones_t = wpool.tile([1, C], F32, tag="ones_t")
        nc.vector.memset(ones_t[:], 1.0)

        # ---- per-batch stages ----
        def p1_dma(b, st, ch=None):
            ch = ch or DCH
            xr = xr_pool.tile([C, L + 2], BF16, tag="xr")
            st["xr"] = xr
            nc.vector.memset(xr[:, 0:1], 0.0)
            nc.vector.memset(xr[:, L + 1:L + 2], 0.0)
            for q in range(L // ch):
                nc.sync.dma_start(xr[:, 1 + q * ch:1 + (q + 1) * ch],
                                  x_ap[b, :, q * ch:(q + 1) * ch])

        def p1_abs(b, st):
            # sum|x| = sum(max(x,0)) - sum(min(x,0)); two 4x TS passes per half
            xr = st["xr"]
            sp = st_pool.tile([C, 4], F32, tag="sp")
            st["sp"] = sp
            if abs_eng == "dve":
                for h in range(2):
                    xs = xr[:, 1 + h * 4096:1 + (h + 1) * 4096]
                    j = junk_pool.tile([C, 4096], BF16, tag="junkA")
                    nc.vector.tensor_scalar(j[:], xs, 0.0, None, ALU.max, ALU.add,
                                            accum_out=sp[:, h:h + 1])
                    j2 = junk_pool.tile([C, 4096], BF16, tag="junkA")
                    nc.vector.tensor_scalar(j2[:], xs, 0.0, None, ALU.min, ALU.add,
                                            accum_out=sp[:, 2 + h:3 + h])
            else:
                for h in range(2):
                    xs = xr[:, 1 + h * 4096:1 + (h + 1) * 4096]
                    j = junk_pool.tile([C, 4096], BF16, tag="junkA")
                    nc.scalar.activation(j[:], xs, AF.Abs, accum_out=sp[:, h:h + 1])
                nc.vector.memset(sp[:, 2:4], 0.0)

        def mlp(b, st):
            sp = st["sp"]
            if abs_eng == "dve":
                spos = st_pool.tile([C, 1], F32, tag="spos")
                nc.vector.tensor_reduce(spos[:], sp[:, 0:2], AX.X, ALU.add)
                sneg = st_pool.tile([C, 1], F32, tag="sneg")
                nc.vector.tensor_reduce(sneg[:], sp[:, 2:4], AX.X, ALU.add)
                sabs = st_pool.tile([C, 1], F32, tag="sabs")
                nc.vector.tensor_tensor(sabs[:], spos[:], sneg[:], ALU.subtract)
            else:
                sabs = st_pool.tile([C, 1], F32, tag="sabs")
                nc.vector.tensor_reduce(sabs[:], sp[:, 0:2], AX.X, ALU.add)
            h_ps = s_psp.tile([CQ, 1], F32, tag="s_ps")
            nc.tensor.matmul(h_ps[:], wfc1_t[:], sabs[:], start=True, stop=True)
            h_t = st_pool.tile([CQ, 1], F32, tag="h_t")
            nc.scalar.activation(h_t[:], h_ps[:], AF.Relu, bias=b1e_t[:], scale=1.0)
            y_ps = s_psp.tile([C, 1], F32, tag="s_ps")
            nc.tensor.matmul(y_ps[:], wfc2_t[:], h_t[:], start=True, stop=True)
            x12 = st_pool.tile([C, 1], F32, tag="x12")
            nc.scalar.activation(x12[:], y_ps[:], AF.Sigmoid, bias=b2_t[:], scale=1.0)
            tpos = st_pool.tile([C, 1], F32, tag="tpos")
            nc.vector.scalar_tensor_tensor(tpos[:], sabs[:], 1.0 / L, x12[:], ALU.mult, ALU.mult)
            neg2t = st_pool.tile([C, 1], F32, tag="neg2t")
            nc.vector.scalar_tensor_tensor(neg2t[:], sabs[:], -2.0 / L, x12[:], ALU.mult, ALU.mult)
            st["tpos"], st["neg2t"] = tpos, neg2t

        def p2_init(b, st):
            x1 = x1_pool.tile([C, L], BF16, tag="x1")
            st["x1"] = x1
            ssum_p = st_pool.tile([C, 2], F32, tag="ssum_p")
            st["ssum_p"] = ssum_p
            smax_p = st_pool.tile([C, 2], F32, tag="smax_p")
            st["smax_p"] = smax_p

        def p2_chunk(b, st, p):
            xr, tpos, neg2t = st["xr"], st["tpos"], st["neg2t"]
            x1 = st["x1"]
            base = 1 + p * UCH
            u_ps = u_psp.tile([C, UCH], F32, tag="u_ps")
            for j in range(UCH // 512):
                nc.tensor.matmul(u_ps[:, j * 512:(j + 1) * 512], w_u_t[:],
                                 xr[:, base + j * 512:base + (j + 1) * 512],
                                 start=True, stop=True)
            # up = u + T  (ScalarE evacuates PSUM, converts to bf16)
            up = up_pool.tile([C, UCH], BF16, tag="up")
            nc.scalar.activation(up[:], u_ps[:], AF.Identity, bias=tpos[:], scale=1.0)
            # um2 = up - 2T on the otherwise-idle GpSimd engine (plain
            # tensor_scalar is the only elementwise op walrus accepts on Pool)
            um2 = gj_pool.tile([C, UCH], BF16, tag="um2")
            if um2_eng == "gpsimd":
                nc.gpsimd.tensor_scalar(um2[:], up[:], neg2t[:], None, ALU.add)
            else:
                nc.vector.tensor_scalar(um2[:], up[:], neg2t[:], None, ALU.add)
            # m = min(up, x);  x1 = max(um2, m) = clamp(x, u-T, u+T)
            m_t = m_pool.tile([C, UCH], BF16, tag="m_t")
            nc.vector.tensor_tensor(m_t[:], up[:], xr[:, base:base + UCH], ALU.min)
            nc.vector.tensor_tensor(x1[:, p * UCH:(p + 1) * UCH], um2[:], m_t[:], ALU.max)

        def p2_stats(b, st):
            # batch-wide sum/max of x1 via 4x-mode tensor_scalar accumulates
            x1, ssum_p, smax_p = st["x1"], st["ssum_p"], st["smax_p"]
            for h in range(2):
                xs = x1[:, h * 4096:(h + 1) * 4096]
                j = junk_pool.tile([C, 4096], BF16, tag="junkA")
                nc.vector.tensor_scalar(j[:], xs, 0.0, None, ALU.add, ALU.add,
                                        accum_out=ssum_p[:, h:h + 1])
                j2 = junk_pool.tile([C, 4096], BF16, tag="junkA")
                nc.vector.tensor_scalar(j2[:], xs, 0.0, None, ALU.add, ALU.max,
                                        accum_out=smax_p[:, h:h + 1])

        def ach(b, st):
            # a = sigmoid(am @ mean(x1) + ax @ max(x1)), computed in ROW form:
            # lg_row[1,C] = s_x1^T @ am^T + mx^T @ ax^T  (lhsT = per-batch [C,1]
            # stats vector, moving = static [C,C] banded matrix) — avoids the
            # transpose round-trip through PSUM.
            s_x1 = st_pool.tile([C, 1], F32, tag="s_x1")
            nc.vector.tensor_reduce(s_x1[:], st["ssum_p"][:], AX.X, ALU.add)
            mx = st_pool.tile([C, 1], F32, tag="mx")
            nc.vector.tensor_reduce(mx[:], st["smax_p"][:], AX.X, ALU.max)
            lg_ps = s_psp.tile([1, C], F32, tag="s_ps")
            nc.tensor.matmul(lg_ps[:], s_x1[:], wam_t[:], start=True, stop=False)
            nc.tensor.matmul(lg_ps[:], mx[:], wax_t[:], start=False, stop=True)
            arow = row_pool.tile([1, C], F32, tag="arow")
            nc.scalar.activation(arow[:], lg_ps[:], AF.Sigmoid)
            bc_ps = s_psp.tile([C, C], F32, tag="s_ps")
            nc.tensor.matmul(bc_ps[:], ones_t[:], arow[:], start=True, stop=True)
            w2a = w2a_pool.tile([C, C], BF16, tag="w2a")
            nc.vector.tensor_tensor(w2a[:], w2t_t[:], bc_ps[:], ALU.mult)
            st["w2a"] = w2a

        def p3_group(b, st, g):
            xr, x1, w2a = st["xr"], st["x1"], st["w2a"]
            ot = out_pool.tile([C, OG], BF16, tag="ot")
            for hh in range(OG // OT):
                b0 = g * OG + hh * OT
                o_ps = o_psp.tile([C, OT], F32, tag="o_ps")
                nc.tensor.matmul(o_ps[:], wsc_t[0][:], xr[:, b0:b0 + OT], start=True, stop=False)
                nc.tensor.matmul(o_ps[:], wsc_t[1][:], xr[:, b0 + 1:b0 + 1 + OT], start=False, stop=False)
                nc.tensor.matmul(o_ps[:], wsc_t[2][:], xr[:, b0 + 2:b0 + 2 + OT], start=False, stop=False)
                nc.tensor.matmul(o_ps[:], w2a[:], x1[:, b0:b0 + OT], start=False, stop=True)
                nc.scalar.activation(ot[:, hh * OT:(hh + 1) * OT], o_ps[:],
                                     AF.Relu, bias=t2_t[:], scale=1.0)
            nc.sync.dma_start(out_ap[b, :, g * OG:(g + 1) * OG], ot[:])

        loop_cm = tc.For_i(0, loop_reps, 1) if loop_reps else None
        if loop_cm is not None:
            loop_cm.__enter__()

        # 4-deep software pipeline: step s runs p1(s) | mlp+p2(s-1) | ach(s-2)
        # | p3(s-3).  ach's serial cross-engine chain has a full step of slack
        # on both sides (inputs ready in step s-1, w2a consumed in step s+1).
        seq = [b for _ in range(reps) for b in range(NB)]
        states = {}
        NG = L // OG
        for s in range(len(seq) + 3):
            if 1 <= s <= len(seq):
                mlp(seq[s - 1], states[s - 1])
                p2_init(seq[s - 1], states[s - 1])
            if 2 <= s <= len(seq) + 1:
                ach(seq[s - 2], states[s - 2])
            if s < len(seq):
                states[s] = {}
                p1_dma(seq[s], states[s], ch=(1024 if s == 0 else None))
                p1_abs(seq[s], states[s])
            # interleave p3(s-3) output groups with p2(s-1) chunks so the PE
            # queue alternates between the two streams
            for k in range(max(NG, NCH)):
                if 3 <= s and k < NG:
                    p3_group(seq[s - 3], states[s - 3], k)
                if 1 <= s <= len(seq) and k < NCH:
                    p2_chunk(seq[s - 1], states[s - 1], k)
            if 1 <= s <= len(seq):
                p2_stats(seq[s - 1], states[s - 1])
            if 3 <= s:
                del states[s - 3]

        if loop_cm is not None:
            loop_cm.__exit__(None, None, None)

    nc.compile()
    _BUILD_CACHE[key] = nc
    return nc


def _host_weights(w_fc1, b_fc1, bn1_g, bn1_b, bn1_rm, bn1_rv, w_fc2, b_fc2,
                  w1, w2, w_sp, w_sc, bn2_g, bn2_b, bn2_rm, bn2_rv):
    f = np.float32
    s1 = (bn1_g / np.sqrt(bn1_rv + EPS)).astype(f)
    t1 = (bn1_b - bn1_rm * s1).astype(f)
    wfc1 = np.ascontiguousarray(((w_fc1 * s1[:, None]) / L).T, dtype=f)      # [C, CQ]
    b1e = np.ascontiguousarray((b_fc1 * s1 + t1)[:, None], dtype=f)          # [CQ, 1]
    wfc2 = np.ascontiguousarray(w_fc2.T, dtype=f)                            # [CQ, C]
    b2 = np.ascontiguousarray(b_fc2[:, None], dtype=f)                       # [C, 1]
    w_u = np.ascontiguousarray((np.eye(C, dtype=f) + w1[:, :, 0]).T, dtype=NPBF16)
    w2t = np.ascontiguousarray(w2[:, :, 0].T, dtype=NPBF16)
    s2 = (bn2_g / np.sqrt(bn2_rv + EPS)).astype(f)
    t2 = np.ascontiguousarray((bn2_b - bn2_rm * s2)[:, None], dtype=f)
    wsc = [np.ascontiguousarray((w_sc[:, :, k] * s2[:, None]).T, dtype=NPBF16) for k in range(3)]
    # banded matrices for the channel-axis conv of [mean, max] rows
    wm = (w_sp[0, 0, :] / L).astype(f)
    wx = w_sp[0, 1, :].astype(f)
    am = (wm[0] * np.eye(C, k=-1) + wm[1] * np.eye(C) + wm[2] * np.eye(C, k=1)).astype(f)
    ax = (wx[0] * np.eye(C, k=-1) + wx[1] * np.eye(C) + wx[2] * np.eye(C, k=1)).astype(f)
    return {
        "w_u": w_u, "wsc0": wsc[0], "wsc1": wsc[1], "wsc2": wsc[2],
        "w2t": w2t, "wfc1": wfc1, "b1e": b1e, "wfc2": wfc2, "b2": b2,
        "t2": t2,
        "wam": np.ascontiguousarray(am.T), "wax": np.ascontiguousarray(ax.T),
    }


def _core_inputs(x, wd, c):
    m = dict(wd)
    m["x_dram"] = np.ascontiguousarray(x[c * NB:(c + 1) * NB].astype(NPBF16))
    return m


def kernel(x, w_fc1, b_fc1, bn1_g, bn1_b, bn1_rm, bn1_rv, w_fc2, b_fc2,
           w1, w2, w_sp, w_sc, bn2_g, bn2_b, bn2_rm, bn2_rv):
    x = np.asarray(x, dtype=np.float32)
    wd = _host_weights(np.asarray(w_fc1, np.float32), np.asarray(b_fc1, np.float32),
                       np.asarray(bn1_g, np.float32), np.asarray(bn1_b, np.float32),
                       np.asarray(bn1_rm, np.float32), np.asarray(bn1_rv, np.float32),
                       np.asarray(w_fc2, np.float32), np.asarray(b_fc2, np.float32),
                       np.asarray(w1, np.float32), np.asarray(w2, np.float32),
                       np.asarray(w_sp, np.float32), np.asarray(w_sc, np.float32),
                       np.asarray(bn2_g, np.float32), np.asarray(bn2_b, np.float32),
                       np.asarray(bn2_rm, np.float32), np.asarray(bn2_rv, np.float32))

    nc = _build()
    in_maps = [_core_inputs(x, wd, c) for c in range(N_CORES)]
    res = bass_utils.run_bass_kernel_spmd(nc, in_maps, core_ids=list(range(N_CORES)))
    out = np.concatenate([np.asarray(res.results[c]["out_dram"]).astype(np.float32)
                          for c in range(N_CORES)], axis=0)
    return out


# revision 13
# speedup vs baseline: 2.4935x; 2.4935x over previous
import os
import sys

if "/opt/trn_rl_repo" not in sys.path:
    sys.path.insert(0, "/opt/trn_rl_repo")

import numpy as np
import ml_dtypes
from contextlib import ExitStack

import concourse.tile as tile
from concourse import bacc, mybir
from concourse import bass_utils

F32 = mybir.dt.float32
BF16 = mybir.dt.bfloat16
AF = mybir.ActivationFunctionType
ALU = mybir.AluOpType
AX = mybir.AxisListType
NPBF16 = ml_dtypes.bfloat16

B, C, L = 32, 128, 8192
N_CORES = 8
NB = B // N_CORES          # batches per core
CQ = C // 4
EPS = 1e-5
UCH = 1024                 # u-chunk width (PSUM fp32, 2 banks)
NCH = L // UCH             # u-chunks per batch
DCH = 2048                 # input DMA chunk width
OT = 512                   # p3 matmul tile (1 PSUM bank)
OG = 1024                  # output DMA group width

_BUILD_CACHE = {}


def _build(reps=1, loop_reps=0):
    key = (reps, loop_reps)
    if key in _BUILD_CACHE:
        return _BUILD_CACHE[key]

    nc = bacc.Bacc("TRN2", target_bir_lowering=False, debug=False)

    x_ap = nc.dram_tensor("x_dram", [NB, C, L], BF16, kind="ExternalInput").ap()
    w_u_ap = nc.dram_tensor("w_u", [C, C], BF16, kind="ExternalInput").ap()
    wsc_aps = [nc.dram_tensor(f"wsc{k}", [C, C], BF16, kind="ExternalInput").ap() for k in range(3)]
    w2t_ap = nc.dram_tensor("w2t", [C, C], BF16, kind="ExternalInput").ap()
    wfc1_ap = nc.dram_tensor("wfc1", [C, CQ], F32, kind="ExternalInput").ap()
    b1e_ap = nc.dram_tensor("b1e", [CQ, 1], F32, kind="ExternalInput").ap()
    wfc2_ap = nc.dram_tensor("wfc2", [CQ, C], F32, kind="ExternalInput").ap()
    b2_ap = nc.dram_tensor("b2", [C, 1], F32, kind="ExternalInput").ap()
    t2_ap = nc.dram_tensor("t2", [C, 1], F32, kind="ExternalInput").ap()
    wam_ap = nc.dram_tensor("wam", [C, C], F32, kind="ExternalInput").ap()
    wax_ap = nc.dram_tensor("wax", [C, C], F32, kind="ExternalInput").ap()
    out_ap = nc.dram_tensor("out_dram", [NB, C, L], BF16, kind="ExternalOutput").ap()

    env = os.environ.get
    xr_bufs = int(env("K_XRBUFS", "4"))
    x1_bufs = int(env("K_X1BUFS", "3"))
    up_bufs = int(env("K_UPBUFS", "3"))
    m_bufs = int(env("K_MBUFS", "3"))
    ub = int(env("K_UBUFS", "2"))
    ob = int(env("K_OBUFS", "3"))
    um2_eng = env("K_UM2ENG", "gpsimd")     # gpsimd | dve
    x1_mode = env("K_X1MODE", "tt")         # tt | stt
    abs_eng = env("K_ABSENG", "dve")        # dve | act

    with tile.TileContext(nc) as tc, ExitStack() as ctx:
        wpool = ctx.enter_context(tc.tile_pool(name="wpool", bufs=1))
        xr_pool = ctx.enter_context(tc.tile_pool(name="xr", bufs=xr_bufs))
        x1_pool = ctx.enter_context(tc.tile_pool(name="x1", bufs=x1_bufs))
        up_pool = ctx.enter_context(tc.tile_pool(name="up", bufs=up_bufs))
        m_pool = ctx.enter_context(tc.tile_pool(name="mtile", bufs=m_bufs))
        junk_pool = ctx.enter_context(tc.tile_pool(name="junk", bufs=2))
        gj_pool = ctx.enter_context(tc.tile_pool(name="gjunk", bufs=2))
        out_pool = ctx.enter_context(tc.tile_pool(name="ot", bufs=3))
        st_pool = ctx.enter_context(tc.tile_pool(name="stats", bufs=2))
        row_pool = ctx.enter_context(tc.tile_pool(name="rows", bufs=2))
        w2a_pool = ctx.enter_context(tc.tile_pool(name="w2a", bufs=3))
        u_psp = ctx.enter_context(tc.tile_pool(name="u_ps", bufs=ub, space="PSUM"))
        o_psp = ctx.enter_context(tc.tile_pool(name="o_ps", bufs=ob, space="PSUM"))
        s_psp = ctx.enter_context(tc.tile_pool(name="s_ps", bufs=int(env("K_SBUFS", "1")), space="PSUM"))

        # ---- load weights (once) ----
        def wload(nm, ap, shape, dt):
            t = wpool.tile(shape, dt, tag=nm)
            nc.sync.dma_start(t[:], ap[:])
            return t

        w_u_t = wload("w_u_t", w_u_ap, [C, C], BF16)
        wsc_t = [wload(f"wsc{k}_t", wsc_aps[k], [C, C], BF16) for k in range(3)]
        w2t_t = wload("w2t_t", w2t_ap, [C, C], BF16)
        wfc1_t = wload("wfc1_t", wfc1_ap, [C, CQ], F32)
        b1e_t = wload("b1e_t", b1e_ap, [CQ, 1], F32)
        wfc2_t = wload("wfc2_t", wfc2_ap, [CQ, C], F32)
        b2_t = wload("b2_t", b2_ap, [C, 1], F32)
        t2_t = wload("t2_t", t2_ap, [C, 1], F32)
        wam_t = wload("wam_t", wam_ap, [C, C], F32)
        wax_t = wload("wax_t", wax_ap, [C, C], F32)
        ones_t = wpool.tile([1, C], F32, tag="ones_t")
        nc.vector.memset(ones_t[:], 1.0)

        # ---- per-batch stages ----
        def p1_dma(b, st, ch=None):
            ch = ch or DCH
            xr = xr_pool.tile([C, L + 2], BF16, tag="xr")
            st["xr"] = xr
            nc.vector.memset(xr[:, 0:1], 0.0)
            nc.vector.memset(xr[:, L + 1:L + 2], 0.0)
            for q in range(L // ch):
                nc.sync.dma_start(xr[:, 1 + q * ch:1 + (q + 1) * ch],
                                  x_ap[b, :, q * ch:(q + 1) * ch])

        def p1_abs(b, st):
            # sum|x| = sum(max(x,0)) - sum(min(x,0)); two 4x TS passes per half
            xr = st["xr"]
            sp = st_pool.tile([C, 4], F32, tag="sp")
            st["sp"] = sp
            if abs_eng == "dve":
                for h in range(2):
                    xs = xr[:, 1 + h * 4096:1 + (h + 1) * 4096]
                    j = junk_pool.tile([C, 4096], BF16, tag="junkA")
                    nc.vector.tensor_scalar(j[:], xs, 0.0, None, ALU.max, ALU.add,
                                            accum_out=sp[:, h:h + 1])
                    j2 = junk_pool.tile([C, 4096], BF16, tag="junkA")
                    nc.vector.tensor_scalar(j2[:], xs, 0.0, None, ALU.min, ALU.add,
                                            accum_out=sp[:, 2 + h:3 + h])
            else:
                for h in range(2):
                    xs = xr[:, 1 + h * 4096:1 + (h + 1) * 4096]
                    j = junk_pool.tile([C, 4096], BF16, tag="junkA")
                    nc.scalar.activation(j[:], xs, AF.Abs, accum_out=sp[:, h:h + 1])
                nc.vector.memset(sp[:, 2:4], 0.0)

        def mlp(b, st):
            sp = st["sp"]
            if abs_eng == "dve":
                spos = st_pool.tile([C, 1], F32, tag="spos")
                nc.vector.tensor_reduce(spos[:], sp[:, 0:2], AX.X, ALU.add)
                sneg = st_pool.tile([C, 1], F32, tag="sneg")
                nc.vector.tensor_reduce(sneg[:], sp[:, 2:4], AX.X, ALU.add)
                sabs = st_pool.tile([C, 1], F32, tag="sabs")
                nc.vector.tensor_tensor(sabs[:], spos[:], sneg[:], ALU.subtract)
            else:
                sabs = st_pool.tile([C, 1], F32, tag="sabs")
                nc.vector.tensor_reduce(sabs[:], sp[:, 0:2], AX.X, ALU.add)
            h_ps = s_psp.tile([CQ, 1], F32, tag="s_ps")
            nc.tensor.matmul(h_ps[:], wfc1_t[:], sabs[:], start=True, stop=True)
            h_t = st_pool.tile([CQ, 1], F32, tag="h_t")
            nc.scalar.activation(h_t[:], h_ps[:], AF.Relu, bias=b1e_t[:], scale=1.0)
            y_ps = s_psp.tile([C, 1], F32, tag="s_ps")
            nc.tensor.matmul(y_ps[:], wfc2_t[:], h_t[:], start=True, stop=True)
            x12 = st_pool.tile([C, 1], F32, tag="x12")
            nc.scalar.activation(x12[:], y_ps[:], AF.Sigmoid, bias=b2_t[:], scale=1.0)
            tpos = st_pool.tile([C, 1], F32, tag="tpos")
            nc.vector.scalar_tensor_tensor(tpos[:], sabs[:], 1.0 / L, x12[:], ALU.mult, ALU.mult)
            neg2t = st_pool.tile([C, 1], F32, tag="neg2t")
            nc.vector.scalar_tensor_tensor(neg2t[:], sabs[:], -2.0 / L, x12[:], ALU.mult, ALU.mult)
            st["tpos"], st["neg2t"] = tpos, neg2t

        def p2_init(b, st):
            x1 = x1_pool.tile([C, L], BF16, tag="x1")
            st["x1"] = x1
            ssum_p = st_pool.tile([C, 2], F32, tag="ssum_p")
            st["ssum_p"] = ssum_p
            smax_p = st_pool.tile([C, 2], F32, tag="smax_p")
            st["smax_p"] = smax_p
            if x1_mode == "stt":
                st["ssum_c"] = st_pool.tile([C, NCH], F32, tag="ssum_c")

        def p2_chunk(b, st, p):
            xr, tpos, neg2t = st["xr"], st["tpos"], st["neg2t"]
            x1 = st["x1"]
            base = 1 + p * UCH
            u_ps = u_psp.tile([C, UCH], F32, tag="u_ps")
            for j in range(UCH // 512):
                nc.tensor.matmul(u_ps[:, j * 512:(j + 1) * 512], w_u_t[:],
                                 xr[:, base + j * 512:base + (j + 1) * 512],
                                 start=True, stop=True)
            # up = u + T  (ScalarE evacuates PSUM, converts to bf16)
            up = up_pool.tile([C, UCH], BF16, tag="up")
            nc.scalar.activation(up[:], u_ps[:], AF.Identity, bias=tpos[:], scale=1.0)
            # m = min(up, x);  x1 = max(up - 2T, m) = clamp(x, u-T, u+T)
            m_t = m_pool.tile([C, UCH], BF16, tag="m_t")
            nc.vector.tensor_tensor(m_t[:], up[:], xr[:, base:base + UCH], ALU.min)
            if x1_mode == "stt":
                # 1x-rate STT but carries the x1 row-sum accumulation for free
                nc.vector.scalar_tensor_tensor(x1[:, p * UCH:(p + 1) * UCH], up[:],
                                               neg2t[:], m_t[:], ALU.add, ALU.max,
                                               accum_out=st["ssum_c"][:, p:p + 1])
            else:
                um2 = gj_pool.tile([C, UCH], BF16, tag="um2")
                if um2_eng == "gpsimd":
                    nc.gpsimd.tensor_scalar(um2[:], up[:], neg2t[:], None, ALU.add)
                else:
                    nc.vector.tensor_scalar(um2[:], up[:], neg2t[:], None, ALU.add)
                nc.vector.tensor_tensor(x1[:, p * UCH:(p + 1) * UCH], um2[:], m_t[:], ALU.max)

        def p2_stats(b, st):
            # batch-wide sum/max of x1 via 4x-mode tensor_scalar accumulates
            x1, ssum_p, smax_p = st["x1"], st["ssum_p"], st["smax_p"]
            for h in range(2):
                xs = x1[:, h * 4096:(h + 1) * 4096]
                if x1_mode != "stt":
                    j = junk_pool.tile([C, 4096], BF16, tag="junkA")
                    nc.vector.tensor_scalar(j[:], xs, 0.0, None, ALU.add, ALU.add,
                                            accum_out=ssum_p[:, h:h + 1])
                j2 = junk_pool.tile([C, 4096], BF16, tag="junkA")
                nc.vector.tensor_scalar(j2[:], xs, 0.0, None, ALU.add, ALU.max,
                                        accum_out=smax_p[:, h:h + 1])

        def ach(b, st):
            # a = sigmoid(am @ mean(x1) + ax @ max(x1)), computed in ROW form:
            # lg_row[1,C] = s_x1^T @ am^T + mx^T @ ax^T  (lhsT = per-batch [C,1]
            # stats vector, moving = static [C,C] banded matrix) — avoids the
            # transpose round-trip through PSUM.
            s_x1 = st_pool.tile([C, 1], F32, tag="s_x1")
            src_sum = st["ssum_c"] if x1_mode == "stt" else st["ssum_p"]
            nc.vector.tensor_reduce(s_x1[:], src_sum[:], AX.X, ALU.add)
            mx = st_pool.tile([C, 1], F32, tag="mx")
            nc.vector.tensor_reduce(mx[:], st["smax_p"][:], AX.X, ALU.max)
            lg_ps = s_psp.tile([1, C], F32, tag="s_ps")
            nc.tensor.matmul(lg_ps[:], s_x1[:], wam_t[:], start=True, stop=False)
            nc.tensor.matmul(lg_ps[:], mx[:], wax_t[:], start=False, stop=True)
            arow = row_pool.tile([1, C], F32, tag="arow")
            nc.scalar.activation(arow[:], lg_ps[:], AF.Sigmoid)
            bc_ps = s_psp.tile([C, C], F32, tag="s_ps")
            nc.tensor.matmul(bc_ps[:], ones_t[:], arow[:], start=True, stop=True)
            w2a = w2a_pool.tile([C, C], BF16, tag="w2a")
            nc.vector.tensor_tensor(w2a[:], w2t_t[:], bc_ps[:], ALU.mult)
            st["w2a"] = w2a

        def p3_group(b, st, g):
            xr, x1, w2a = st["xr"], st["x1"], st["w2a"]
            ot = out_pool.tile([C, OG], BF16, tag="ot")
            for hh in range(OG // OT):
                b0 = g * OG + hh * OT
                o_ps = o_psp.tile([C, OT], F32, tag="o_ps")
                nc.tensor.matmul(o_ps[:], wsc_t[0][:], xr[:, b0:b0 + OT], start=True, stop=False)
                nc.tensor.matmul(o_ps[:], wsc_t[1][:], xr[:, b0 + 1:b0 + 1 + OT], start=False, stop=False)
                nc.tensor.matmul(o_ps[:], wsc_t[2][:], xr[:, b0 + 2:b0 + 2 + OT], start=False, stop=False)
                nc.tensor.matmul(o_ps[:], w2a[:], x1[:, b0:b0 + OT], start=False, stop=True)
                nc.scalar.activation(ot[:, hh * OT:(hh + 1) * OT], o_ps[:],
                                     AF.Relu, bias=t2_t[:], scale=1.0)
            nc.sync.dma_start(out_ap[b, :, g * OG:(g + 1) * OG], ot[:])

        loop_cm = tc.For_i(0, loop_reps, 1) if loop_reps else None
        if loop_cm is not None:
            loop_cm.__enter__()

        # 4-deep software pipeline: step s runs p1(s) | mlp+p2(s-1) | ach(s-2)
        # | p3(s-3).  ach's serial cross-engine chain has a full step of slack
        # on both sides (inputs ready in step s-1, w2a consumed in step s+1).
        seq = [b for _ in range(reps) for b in range(NB)]
        states = {}
        NG = L // OG
        for s in range(len(seq) + 3):
            if 1 <= s <= len(seq):
                mlp(seq[s - 1], states[s - 1])
                p2_init(seq[s - 1], states[s - 1])
            if 2 <= s <= len(seq) + 1:
                ach(seq[s - 2], states[s - 2])
            if s < len(seq):
                states[s] = {}
                p1_dma(seq[s], states[s], ch=(1024 if s == 0 else None))
                p1_abs(seq[s], states[s])
            # interleave p3(s-3) output groups with p2(s-1) chunks so the PE
            # queue alternates between the two streams
            for k in range(max(NG, NCH)):
                if 3 <= s and k < NG:
                    p3_group(seq[s - 3], states[s - 3], k)
                if 1 <= s <= len(seq) and k < NCH:
                    p2_chunk(seq[s - 1], states[s - 1], k)
            if 1 <= s <= len(seq):
                p2_stats(seq[s - 1], states[s - 1])
            if 3 <= s:
                del states[s - 3]

        if loop_cm is not None:
            loop_cm.__exit__(None, None, None)

    nc.compile()
    _BUILD_CACHE[key] = nc
    return nc


def _host_weights(w_fc1, b_fc1, bn1_g, bn1_b, bn1_rm, bn1_rv, w_fc2, b_fc2,
                  w1, w2, w_sp, w_sc, bn2_g, bn2_b, bn2_rm, bn2_rv):
    f = np.float32
    s1 = (bn1_g / np.sqrt(bn1_rv + EPS)).astype(f)
    t1 = (bn1_b - bn1_rm * s1).astype(f)
    wfc1 = np.ascontiguousarray(((w_fc1 * s1[:, None]) / L).T, dtype=f)      # [C, CQ]
    b1e = np.ascontiguousarray((b_fc1 * s1 + t1)[:, None], dtype=f)          # [CQ, 1]
    wfc2 = np.ascontiguousarray(w_fc2.T, dtype=f)                            # [CQ, C]
    b2 = np.ascontiguousarray(b_fc2[:, None], dtype=f)                       # [C, 1]
    w_u = np.ascontiguousarray((np.eye(C, dtype=f) + w1[:, :, 0]).T, dtype=NPBF16)
    w2t = np.ascontiguousarray(w2[:, :, 0].T, dtype=NPBF16)
    s2 = (bn2_g / np.sqrt(bn2_rv + EPS)).astype(f)
    t2 = np.ascontiguousarray((bn2_b - bn2_rm * s2)[:, None], dtype=f)
    wsc = [np.ascontiguousarray((w_sc[:, :, k] * s2[:, None]).T, dtype=NPBF16) for k in range(3)]
    # banded matrices for the channel-axis conv of [mean, max] rows
    wm = (w_sp[0, 0, :] / L).astype(f)
    wx = w_sp[0, 1, :].astype(f)
    am = (wm[0] * np.eye(C, k=-1) + wm[1] * np.eye(C) + wm[2] * np.eye(C, k=1)).astype(f)
    ax = (wx[0] * np.eye(C, k=-1) + wx[1] * np.eye(C) + wx[2] * np.eye(C, k=1)).astype(f)
    return {
        "w_u": w_u, "wsc0": wsc[0], "wsc1": wsc[1], "wsc2": wsc[2],
        "w2t": w2t, "wfc1": wfc1, "b1e": b1e, "wfc2": wfc2, "b2": b2,
        "t2": t2,
        "wam": np.ascontiguousarray(am.T), "wax": np.ascontiguousarray(ax.T),
    }


def _core_inputs(x, wd, c):
    m = dict(wd)
    m["x_dram"] = np.ascontiguousarray(x[c * NB:(c + 1) * NB].astype(NPBF16))
    return m


def kernel(x, w_fc1, b_fc1, bn1_g, bn1_b, bn1_rm, bn1_rv, w_fc2, b_fc2,
           w1, w2, w_sp, w_sc, bn2_g, bn2_b, bn2_rm, bn2_rv):
    x = np.asarray(x, dtype=np.float32)
    wd = _host_weights(np.asarray(w_fc1, np.float32), np.asarray(b_fc1, np.float32),
                       np.asarray(bn1_g, np.float32), np.asarray(bn1_b, np.float32),
                       np.asarray(bn1_rm, np.float32), np.asarray(bn1_rv, np.float32),
                       np.asarray(w_fc2, np.float32), np.asarray(b_fc2, np.float32),
                       np.asarray(w1, np.float32), np.asarray(w2, np.float32),
                       np.asarray(w_sp, np.float32), np.asarray(w_sc, np.float32),
                       np.asarray(bn2_g, np.float32), np.asarray(bn2_b, np.float32),
                       np.asarray(bn2_rm, np.float32), np.asarray(bn2_rv, np.float32))

    nc = _build()
    in_maps = [_core_inputs(x, wd, c) for c in range(N_CORES)]
    res = bass_utils.run_bass_kernel_spmd(nc, in_maps, core_ids=list(range(N_CORES)))
    out = np.concatenate([np.asarray(res.results[c]["out_dram"]).astype(np.float32)
                          for c in range(N_CORES)], axis=0)
    return out


# revision 14
# speedup vs baseline: 2.6815x; 1.0754x over previous
import os
import sys

if "/opt/trn_rl_repo" not in sys.path:
    sys.path.insert(0, "/opt/trn_rl_repo")

import numpy as np
import ml_dtypes
from contextlib import ExitStack

import concourse.tile as tile
from concourse import bacc, mybir
from concourse import bass_utils

F32 = mybir.dt.float32
BF16 = mybir.dt.bfloat16
AF = mybir.ActivationFunctionType
ALU = mybir.AluOpType
AX = mybir.AxisListType
NPBF16 = ml_dtypes.bfloat16

B, C, L = 32, 128, 8192
N_CORES = 8
NB = B // N_CORES          # batches per core
CQ = C // 4
EPS = 1e-5
UCH = 1024                 # u-chunk width (PSUM fp32, 2 banks)
NCH = L // UCH             # u-chunks per batch
DCH = 2048                 # input DMA chunk width
OT = 512                   # p3 matmul tile (1 PSUM bank)
OG = 1024                  # output DMA group width

_BUILD_CACHE = {}


def _build(reps=1, loop_reps=0):
    key = (reps, loop_reps)
    if key in _BUILD_CACHE:
        return _BUILD_CACHE[key]

    nc = bacc.Bacc("TRN2", target_bir_lowering=False, debug=False)

    x_ap = nc.dram_tensor("x_dram", [NB, C, L], BF16, kind="ExternalInput").ap()
    w_u_ap = nc.dram_tensor("w_u", [C, C], BF16, kind="ExternalInput").ap()
    wsc_aps = [nc.dram_tensor(f"wsc{k}", [C, C], BF16, kind="ExternalInput").ap() for k in range(3)]
    w2t_ap = nc.dram_tensor("w2t", [C, C], BF16, kind="ExternalInput").ap()
    wfc1_ap = nc.dram_tensor("wfc1", [C, CQ], F32, kind="ExternalInput").ap()
    b1e_ap = nc.dram_tensor("b1e", [CQ, 1], F32, kind="ExternalInput").ap()
    wfc2_ap = nc.dram_tensor("wfc2", [CQ, C], F32, kind="ExternalInput").ap()
    b2_ap = nc.dram_tensor("b2", [C, 1], F32, kind="ExternalInput").ap()
    t2_ap = nc.dram_tensor("t2", [C, 1], F32, kind="ExternalInput").ap()
    wam_ap = nc.dram_tensor("wam", [C, C], F32, kind="ExternalInput").ap()
    wax_ap = nc.dram_tensor("wax", [C, C], F32, kind="ExternalInput").ap()
    out_ap = nc.dram_tensor("out_dram", [NB, C, L], BF16, kind="ExternalOutput").ap()

    env = os.environ.get
    xr_bufs = int(env("K_XRBUFS", "4"))
    x1_bufs = int(env("K_X1BUFS", "3"))
    up_bufs = int(env("K_UPBUFS", "3"))
    m_bufs = int(env("K_MBUFS", "3"))
    ub = int(env("K_UBUFS", "2"))
    ob = int(env("K_OBUFS", "3"))
    um2_eng = env("K_UM2ENG", "dve")        # dve | gpsimd (gpsimd ~15x slower on HW)
    x1_mode = env("K_X1MODE", "tt")         # tt | stt
    abs_eng = env("K_ABSENG", "dve")        # dve | act

    with tile.TileContext(nc) as tc, ExitStack() as ctx:
        wpool = ctx.enter_context(tc.tile_pool(name="wpool", bufs=1))
        xr_pool = ctx.enter_context(tc.tile_pool(name="xr", bufs=xr_bufs))
        x1_pool = ctx.enter_context(tc.tile_pool(name="x1", bufs=x1_bufs))
        up_pool = ctx.enter_context(tc.tile_pool(name="up", bufs=up_bufs))
        m_pool = ctx.enter_context(tc.tile_pool(name="mtile", bufs=m_bufs))
        junk_pool = ctx.enter_context(tc.tile_pool(name="junk", bufs=2))
        gj_pool = ctx.enter_context(tc.tile_pool(name="gjunk", bufs=2))
        out_pool = ctx.enter_context(tc.tile_pool(name="ot", bufs=3))
        st_pool = ctx.enter_context(tc.tile_pool(name="stats", bufs=2))
        row_pool = ctx.enter_context(tc.tile_pool(name="rows", bufs=2))
        w2a_pool = ctx.enter_context(tc.tile_pool(name="w2a", bufs=3))
        u_psp = ctx.enter_context(tc.tile_pool(name="u_ps", bufs=ub, space="PSUM"))
        o_psp = ctx.enter_context(tc.tile_pool(name="o_ps", bufs=ob, space="PSUM"))
        s_psp = ctx.enter_context(tc.tile_pool(name="s_ps", bufs=int(env("K_SBUFS", "1")), space="PSUM"))

        # ---- load weights (once) ----
        def wload(nm, ap, shape, dt):
            t = wpool.tile(shape, dt, tag=nm)
            nc.sync.dma_start(t[:], ap[:])
            return t

        w_u_t = wload("w_u_t", w_u_ap, [C, C], BF16)
        wsc_t = [wload(f"wsc{k}_t", wsc_aps[k], [C, C], BF16) for k in range(3)]
        w2t_t = wload("w2t_t", w2t_ap, [C, C], BF16)
        wfc1_t = wload("wfc1_t", wfc1_ap, [C, CQ], F32)
        b1e_t = wload("b1e_t", b1e_ap, [CQ, 1], F32)
        wfc2_t = wload("wfc2_t", wfc2_ap, [CQ, C], F32)
        b2_t = wload("b2_t", b2_ap, [C, 1], F32)
        t2_t = wload("t2_t", t2_ap, [C, 1], F32)
        wam_t = wload("wam_t", wam_ap, [C, C], F32)
        wax_t = wload("wax_t", wax_ap, [C, C], F32)
        ones_t = wpool.tile([1, C], F32, tag="ones_t")
        nc.vector.memset(ones_t[:], 1.0)

        # ---- per-batch stages ----
        def p1_dma(b, st, ch=None):
            ch = ch or DCH
            xr = xr_pool.tile([C, L + 4], BF16, tag="xr")
            st["xr"] = xr
            nc.vector.memset(xr[:, 0:2], 0.0)
            nc.vector.memset(xr[:, L + 2:L + 4], 0.0)
            for q in range(L // ch):
                nc.sync.dma_start(xr[:, 2 + q * ch:2 + (q + 1) * ch],
                                  x_ap[b, :, q * ch:(q + 1) * ch])

        def p1_abs(b, st):
            # sum|x| = sum(max(x,0)) - sum(min(x,0)); two 4x TS passes per half
            xr = st["xr"]
            sp = st_pool.tile([C, 4], F32, tag="sp")
            st["sp"] = sp
            if abs_eng == "dve":
                for h in range(2):
                    xs = xr[:, 2 + h * 4096:2 + (h + 1) * 4096]
                    j = junk_pool.tile([C, 4096], BF16, tag="junkA")
                    nc.vector.tensor_scalar(j[:], xs, 0.0, None, ALU.max, ALU.add,
                                            accum_out=sp[:, h:h + 1])
                    j2 = junk_pool.tile([C, 4096], BF16, tag="junkA")
                    nc.vector.tensor_scalar(j2[:], xs, 0.0, None, ALU.min, ALU.add,
                                            accum_out=sp[:, 2 + h:3 + h])
            else:
                for h in range(2):
                    xs = xr[:, 2 + h * 4096:2 + (h + 1) * 4096]
                    j = junk_pool.tile([C, 4096], BF16, tag="junkA")
                    nc.scalar.activation(j[:], xs, AF.Abs, accum_out=sp[:, h:h + 1])
                nc.vector.memset(sp[:, 2:4], 0.0)

        def mlp(b, st):
            sp = st["sp"]
            if abs_eng == "dve":
                spos = st_pool.tile([C, 1], F32, tag="spos")
                nc.vector.tensor_reduce(spos[:], sp[:, 0:2], AX.X, ALU.add)
                sneg = st_pool.tile([C, 1], F32, tag="sneg")
                nc.vector.tensor_reduce(sneg[:], sp[:, 2:4], AX.X, ALU.add)
                sabs = st_pool.tile([C, 1], F32, tag="sabs")
                nc.vector.tensor_tensor(sabs[:], spos[:], sneg[:], ALU.subtract)
            else:
                sabs = st_pool.tile([C, 1], F32, tag="sabs")
                nc.vector.tensor_reduce(sabs[:], sp[:, 0:2], AX.X, ALU.add)
            h_ps = s_psp.tile([CQ, 1], F32, tag="s_ps")
            nc.tensor.matmul(h_ps[:], wfc1_t[:], sabs[:], start=True, stop=True)
            h_t = st_pool.tile([CQ, 1], F32, tag="h_t")
            nc.scalar.activation(h_t[:], h_ps[:], AF.Relu, bias=b1e_t[:], scale=1.0)
            y_ps = s_psp.tile([C, 1], F32, tag="s_ps")
            nc.tensor.matmul(y_ps[:], wfc2_t[:], h_t[:], start=True, stop=True)
            x12 = st_pool.tile([C, 1], F32, tag="x12")
            nc.scalar.activation(x12[:], y_ps[:], AF.Sigmoid, bias=b2_t[:], scale=1.0)
            tpos = st_pool.tile([C, 1], F32, tag="tpos")
            nc.vector.scalar_tensor_tensor(tpos[:], sabs[:], 1.0 / L, x12[:], ALU.mult, ALU.mult)
            neg2t = st_pool.tile([C, 1], F32, tag="neg2t")
            nc.vector.scalar_tensor_tensor(neg2t[:], sabs[:], -2.0 / L, x12[:], ALU.mult, ALU.mult)
            st["tpos"], st["neg2t"] = tpos, neg2t

        def p2_init(b, st):
            x1 = x1_pool.tile([C, L], BF16, tag="x1")
            st["x1"] = x1
            ssum_p = st_pool.tile([C, 2], F32, tag="ssum_p")
            st["ssum_p"] = ssum_p
            smax_p = st_pool.tile([C, 2], F32, tag="smax_p")
            st["smax_p"] = smax_p
            if x1_mode == "stt":
                st["ssum_c"] = st_pool.tile([C, NCH], F32, tag="ssum_c")

        def p2_chunk(b, st, p):
            xr, tpos, neg2t = st["xr"], st["tpos"], st["neg2t"]
            x1 = st["x1"]
            base = 2 + p * UCH
            u_ps = u_psp.tile([C, UCH], F32, tag="u_ps")
            for j in range(UCH // 512):
                nc.tensor.matmul(u_ps[:, j * 512:(j + 1) * 512], w_u_t[:],
                                 xr[:, base + j * 512:base + (j + 1) * 512],
                                 start=True, stop=True)
            # up = u + T  (ScalarE evacuates PSUM, converts to bf16)
            up = up_pool.tile([C, UCH], BF16, tag="up")
            nc.scalar.activation(up[:], u_ps[:], AF.Identity, bias=tpos[:], scale=1.0)
            # m = min(up, x);  x1 = max(up - 2T, m) = clamp(x, u-T, u+T)
            m_t = m_pool.tile([C, UCH], BF16, tag="m_t")
            nc.vector.tensor_tensor(m_t[:], up[:], xr[:, base:base + UCH], ALU.min)
            if x1_mode == "stt":
                # 1x-rate STT but carries the x1 row-sum accumulation for free
                nc.vector.scalar_tensor_tensor(x1[:, p * UCH:(p + 1) * UCH], up[:],
                                               neg2t[:], m_t[:], ALU.add, ALU.max,
                                               accum_out=st["ssum_c"][:, p:p + 1])
            else:
                um2 = gj_pool.tile([C, UCH], BF16, tag="um2")
                if um2_eng == "gpsimd":
                    nc.gpsimd.tensor_scalar(um2[:], up[:], neg2t[:], None, ALU.add)
                else:
                    nc.vector.tensor_scalar(um2[:], up[:], neg2t[:], None, ALU.add)
                nc.vector.tensor_tensor(x1[:, p * UCH:(p + 1) * UCH], um2[:], m_t[:], ALU.max)

        def p2_stats(b, st):
            # batch-wide sum/max of x1 via 4x-mode tensor_scalar accumulates
            x1, ssum_p, smax_p = st["x1"], st["ssum_p"], st["smax_p"]
            for h in range(2):
                xs = x1[:, h * 4096:(h + 1) * 4096]
                if x1_mode != "stt":
                    j = junk_pool.tile([C, 4096], BF16, tag="junkA")
                    nc.vector.tensor_scalar(j[:], xs, 0.0, None, ALU.add, ALU.add,
                                            accum_out=ssum_p[:, h:h + 1])
                j2 = junk_pool.tile([C, 4096], BF16, tag="junkA")
                nc.vector.tensor_scalar(j2[:], xs, 0.0, None, ALU.add, ALU.max,
                                        accum_out=smax_p[:, h:h + 1])

        def ach(b, st):
            # a = sigmoid(am @ mean(x1) + ax @ max(x1)), computed in ROW form:
            # lg_row[1,C] = s_x1^T @ am^T + mx^T @ ax^T  (lhsT = per-batch [C,1]
            # stats vector, moving = static [C,C] banded matrix) — avoids the
            # transpose round-trip through PSUM.
            s_x1 = st_pool.tile([C, 1], F32, tag="s_x1")
            src_sum = st["ssum_c"] if x1_mode == "stt" else st["ssum_p"]
            nc.vector.tensor_reduce(s_x1[:], src_sum[:], AX.X, ALU.add)
            mx = st_pool.tile([C, 1], F32, tag="mx")
            nc.vector.tensor_reduce(mx[:], st["smax_p"][:], AX.X, ALU.max)
            lg_ps = s_psp.tile([1, C], F32, tag="s_ps")
            nc.tensor.matmul(lg_ps[:], s_x1[:], wam_t[:], start=True, stop=False)
            nc.tensor.matmul(lg_ps[:], mx[:], wax_t[:], start=False, stop=True)
            arow = row_pool.tile([1, C], F32, tag="arow")
            nc.scalar.activation(arow[:], lg_ps[:], AF.Sigmoid)
            bc_ps = s_psp.tile([C, C], F32, tag="s_ps")
            nc.tensor.matmul(bc_ps[:], ones_t[:], arow[:], start=True, stop=True)
            w2a = w2a_pool.tile([C, C], BF16, tag="w2a")
            nc.vector.tensor_tensor(w2a[:], w2t_t[:], bc_ps[:], ALU.mult)
            st["w2a"] = w2a

        def p3_group(b, st, g):
            xr, x1, w2a = st["xr"], st["x1"], st["w2a"]
            ot = out_pool.tile([C, OG], BF16, tag="ot")
            for hh in range(OG // OT):
                b0 = g * OG + hh * OT
                o_ps = o_psp.tile([C, OT], F32, tag="o_ps")
                nc.tensor.matmul(o_ps[:], wsc_t[0][:], xr[:, b0 + 1:b0 + 1 + OT], start=True, stop=False)
                nc.tensor.matmul(o_ps[:], wsc_t[1][:], xr[:, b0 + 2:b0 + 2 + OT], start=False, stop=False)
                nc.tensor.matmul(o_ps[:], wsc_t[2][:], xr[:, b0 + 3:b0 + 3 + OT], start=False, stop=False)
                nc.tensor.matmul(o_ps[:], w2a[:], x1[:, b0:b0 + OT], start=False, stop=True)
                nc.scalar.activation(ot[:, hh * OT:(hh + 1) * OT], o_ps[:],
                                     AF.Relu, bias=t2_t[:], scale=1.0)
            nc.sync.dma_start(out_ap[b, :, g * OG:(g + 1) * OG], ot[:])

        loop_cm = tc.For_i(0, loop_reps, 1) if loop_reps else None
        if loop_cm is not None:
            loop_cm.__enter__()

        # 4-deep software pipeline: step s runs p1(s) | mlp+p2(s-1) | ach(s-2)
        # | p3(s-3).  ach's serial cross-engine chain has a full step of slack
        # on both sides (inputs ready in step s-1, w2a consumed in step s+1).
        seq = [b for _ in range(reps) for b in range(NB)]
        states = {}
        NG = L // OG
        for s in range(len(seq) + 3):
            if 1 <= s <= len(seq):
                mlp(seq[s - 1], states[s - 1])
                p2_init(seq[s - 1], states[s - 1])
            if 2 <= s <= len(seq) + 1:
                ach(seq[s - 2], states[s - 2])
            if s < len(seq):
                states[s] = {}
                p1_dma(seq[s], states[s], ch=(1024 if s == 0 else None))
                p1_abs(seq[s], states[s])
            # interleave p3(s-3) output groups with p2(s-1) chunks so the PE
            # queue alternates between the two streams
            for k in range(max(NG, NCH)):
                if 3 <= s and k < NG:
                    p3_group(seq[s - 3], states[s - 3], k)
                if 1 <= s <= len(seq) and k < NCH:
                    p2_chunk(seq[s - 1], states[s - 1], k)
            if 1 <= s <= len(seq):
                p2_stats(seq[s - 1], states[s - 1])
            if 3 <= s:
                del states[s - 3]

        if loop_cm is not None:
            loop_cm.__exit__(None, None, None)

    nc.compile()
    _BUILD_CACHE[key] = nc
    return nc


def _host_weights(w_fc1, b_fc1, bn1_g, bn1_b, bn1_rm, bn1_rv, w_fc2, b_fc2,
                  w1, w2, w_sp, w_sc, bn2_g, bn2_b, bn2_rm, bn2_rv):
    f = np.float32
    s1 = (bn1_g / np.sqrt(bn1_rv + EPS)).astype(f)
    t1 = (bn1_b - bn1_rm * s1).astype(f)
    wfc1 = np.ascontiguousarray(((w_fc1 * s1[:, None]) / L).T, dtype=f)      # [C, CQ]
    b1e = np.ascontiguousarray((b_fc1 * s1 + t1)[:, None], dtype=f)          # [CQ, 1]
    wfc2 = np.ascontiguousarray(w_fc2.T, dtype=f)                            # [CQ, C]
    b2 = np.ascontiguousarray(b_fc2[:, None], dtype=f)                       # [C, 1]
    w_u = np.ascontiguousarray((np.eye(C, dtype=f) + w1[:, :, 0]).T, dtype=NPBF16)
    w2t = np.ascontiguousarray(w2[:, :, 0].T, dtype=NPBF16)
    s2 = (bn2_g / np.sqrt(bn2_rv + EPS)).astype(f)
    t2 = np.ascontiguousarray((bn2_b - bn2_rm * s2)[:, None], dtype=f)
    wsc = [np.ascontiguousarray((w_sc[:, :, k] * s2[:, None]).T, dtype=NPBF16) for k in range(3)]
    # banded matrices for the channel-axis conv of [mean, max] rows
    wm = (w_sp[0, 0, :] / L).astype(f)
    wx = w_sp[0, 1, :].astype(f)
    am = (wm[0] * np.eye(C, k=-1) + wm[1] * np.eye(C) + wm[2] * np.eye(C, k=1)).astype(f)
    ax = (wx[0] * np.eye(C, k=-1) + wx[1] * np.eye(C) + wx[2] * np.eye(C, k=1)).astype(f)
    return {
        "w_u": w_u, "wsc0": wsc[0], "wsc1": wsc[1], "wsc2": wsc[2],
        "w2t": w2t, "wfc1": wfc1, "b1e": b1e, "wfc2": wfc2, "b2": b2,
        "t2": t2,
        "wam": np.ascontiguousarray(am.T), "wax": np.ascontiguousarray(ax.T),
    }


def _core_inputs(x, wd, c):
    m = dict(wd)
    m["x_dram"] = np.ascontiguousarray(x[c * NB:(c + 1) * NB].astype(NPBF16))
    return m


def kernel(x, w_fc1, b_fc1, bn1_g, bn1_b, bn1_rm, bn1_rv, w_fc2, b_fc2,
           w1, w2, w_sp, w_sc, bn2_g, bn2_b, bn2_rm, bn2_rv):
    x = np.asarray(x, dtype=np.float32)
    wd = _host_weights(np.asarray(w_fc1, np.float32), np.asarray(b_fc1, np.float32),
                       np.asarray(bn1_g, np.float32), np.asarray(bn1_b, np.float32),
                       np.asarray(bn1_rm, np.float32), np.asarray(bn1_rv, np.float32),
                       np.asarray(w_fc2, np.float32), np.asarray(b_fc2, np.float32),
                       np.asarray(w1, np.float32), np.asarray(w2, np.float32),
                       np.asarray(w_sp, np.float32), np.asarray(w_sc, np.float32),
                       np.asarray(bn2_g, np.float32), np.asarray(bn2_b, np.float32),
                       np.asarray(bn2_rm, np.float32), np.asarray(bn2_rv, np.float32))

    nc = _build()
    in_maps = [_core_inputs(x, wd, c) for c in range(N_CORES)]
    res = bass_utils.run_bass_kernel_spmd(nc, in_maps, core_ids=list(range(N_CORES)))
    out = np.concatenate([np.asarray(res.results[c]["out_dram"]).astype(np.float32)
                          for c in range(N_CORES)], axis=0)
    return out
